# revision 5
# baseline (speedup 1.0000x reference)
"""Trainium2 Bass kernel for nn_EpistaticMultiDecoder.

Computes: adapter FFN on (1000,32) features, then for all 1e6 pairs (i,j):
head(LN -> FFN -> Linear(32,1)) of f[i]+f[j], plus ddg[i]+ddg[j].

Strategy: shard rows i across 8 cores (125 rows each). On-chip layout is
"xT4": SBUF tile (128, N) where partition = band*32 + d (4 bands of 32 dims),
band b holds j-tokens [250b, 250b+250). All LayerNorm mean-centering is
algebraic: x @ C32 with C32 = I - ones/32, folded into the surrounding
matmuls (host-precomputed block-diagonal weights). Per-band variance via a
block-diag ones matmul (sum over the 32 partitions of each band, broadcast
back across partitions by the PE itself). LN affine params are folded into
downstream weights. Final Linear + LN affine folded into one (128,4) matmul.
"""

import sys

sys.path.insert(0, "/opt/trn_rl_repo")

import numpy as np

B, L, A, D = 1, 50, 20, 32
M = L * A            # 1000 mutation tokens
NB = 4               # partition bands
BW = M // NB         # 250 tokens per band
NCORES = 8
RPC = M // NCORES    # 125 rows per core
EPS = 1e-5

_CACHE = {}

# packed constant layout: name -> (offset, width) in wpack's free dim
_worder = [("x4", BW), ("wbd1", 128), ("wbd2c", 128), ("ibdc", 128),
           ("onesbd", 128), ("agcbd", 128), ("fbd1p", 128), ("fbd2c", 128),
           ("gbd", 128), ("owp", NB), ("vecs", 6), ("pa", 128), ("pb", 128)]
WOFF = {}
_o = 0
for _n, _w in _worder:
    WOFF[_n] = (_o, _w)
    _o += _w
WPW = _o


def _build_program():
    from concourse import bacc, mybir
    from concourse.tile import TileContext

    fp32 = mybir.dt.float32
    AF = mybir.ActivationFunctionType
    OP = mybir.AluOpType

    nc = bacc.Bacc()

    # ---- DRAM I/O ----
    # All (128, *) constants packed into one tensor -> one DMA -> one
    # semaphore (matmuls may carry at most one extra sync wait on LDWEIGHTS).
    wp = nc.dram_tensor("wpack", [128, WPW], fp32, kind="ExternalInput")
    dp = nc.dram_tensor("dpack", [NB, BW + RPC], fp32, kind="ExternalInput")
    out_d = nc.dram_tensor("out", [RPC, M], fp32, kind="ExternalOutput")

    with TileContext(nc) as tc:
        with (
            tc.tile_pool(name="consts", bufs=1) as consts,
            tc.tile_pool(name="s0", bufs=1) as s0,
            tc.tile_pool(name="lp", bufs=3) as lp,
            tc.tile_pool(name="outp", bufs=3) as outp,
            tc.tile_pool(name="psA", bufs=2, space="PSUM") as psA,
            tc.tile_pool(name="psB", bufs=2, space="PSUM") as psB,
            tc.tile_pool(name="psC", bufs=2, space="PSUM") as psC,
            tc.tile_pool(name="psE", bufs=1, space="PSUM") as psE,
            tc.tile_pool(name="psF", bufs=1, space="PSUM") as psF,
        ):
            wpack = consts.tile_from(wp[:, :], name="wpack")
            dpack = consts.tile_from(dp[:, :], name="dpack")
            ct = {k: wpack[:, o:o + w] for k, (o, w) in WOFF.items()}
            ct["ddgjb"] = dpack[:, 0:BW]
            ct["ddgi"] = dpack[:, BW:BW + RPC]
            x4 = ct["x4"]
            vecs = ct["vecs"]
            ab1r = vecs[:, 0:1]    # tile4(ab1)
            ab2cr = vecs[:, 1:2]   # tile4(ab2 @ C32)
            abtc = vecs[:, 2:3]    # tile4(abt @ C32)
            fb1r = vecs[:, 3:4]    # tile4(fb1 + hbt @ fw1)
            ycb = vecs[:, 4:5]     # tile4((fb2 + hbt) @ C32)
            epsr = vecs[:, 5:6]    # EPS

            # ---- stage 0: adapter FFN over all 1000 tokens (runs once) ----
            ps = psA.tile([128, BW], fp32, tag="psA")
            nc.tensor.matmul(ps, ct["wbd1"], x4, start=True, stop=True)
            hra = s0.tile([128, BW], fp32)
            nc.scalar.activation(hra, ps, AF.Relu, bias=ab1r, scale=1.0)

            ps = psB.tile([128, BW], fp32, tag="psB")
            nc.tensor.matmul(ps, ct["wbd2c"], hra, start=True, stop=False)
            nc.tensor.matmul(ps, ct["ibdc"], x4, start=False, stop=True)
            yca = s0.tile([128, BW], fp32)
            nc.scalar.activation(yca, ps, AF.Identity, bias=ab2cr, scale=1.0)

            sqa = s0.tile([128, BW], fp32)
            nc.vector.tensor_mul(sqa, yca, yca)
            ps = psA.tile([128, BW], fp32, tag="psA")
            nc.tensor.matmul(ps, ct["onesbd"], sqa, start=True, stop=True)
            sa = s0.tile([128, BW], fp32)
            nc.scalar.activation(sa, ps, AF.Sqrt, bias=epsr, scale=1.0 / D)
            ra = s0.tile([128, BW], fp32)
            nc.vector.reciprocal(ra, sa)
            ua = s0.tile([128, BW], fp32)
            nc.vector.tensor_mul(ua, yca, ra)

            ps = psB.tile([128, BW], fp32, tag="psB")
            nc.tensor.matmul(ps, ct["agcbd"], ua, start=True, stop=True)
            fc4 = s0.tile([128, BW], fp32)
            nc.scalar.activation(fc4, ps, AF.Identity, bias=abtc, scale=1.0)

            # per-core row block of fc, band-replicated: (128, 125)
            ps = psA.tile([128, RPC], fp32, tag="psA")
            nc.tensor.matmul(ps, ct["pa"], fc4[:, 0:RPC], start=True, stop=False)
            nc.tensor.matmul(ps, ct["pb"], fc4[:, RPC:BW], start=False, stop=True)
            fcI = s0.tile([128, RPC], fp32)
            nc.scalar.activation(fcI, ps, AF.Copy, bias=0.0, scale=1.0)

            # ---- main loop: 2 output rows per pass ----
            for p in range(63):
                i0 = 2 * p
                n_i = 1 if i0 == RPC - 1 else 2
                N = BW * n_i

                pairc = lp.tile([128, 2 * BW], fp32, tag="pairc")
                for h in range(n_i):
                    nc.vector.tensor_scalar_add(
                        pairc[:, h * BW:(h + 1) * BW], fc4,
                        fcI[:, i0 + h:i0 + h + 1])

                pairsq = lp.tile([128, 2 * BW], fp32, tag="pairsq")
                nc.gpsimd.tensor_mul(pairsq[:, :N], pairc[:, :N], pairc[:, :N])

                psa = psA.tile([128, 2 * BW], fp32, tag="psA")
                nc.tensor.matmul(psa[:, :N], ct["onesbd"], pairsq[:, :N],
                                 start=True, stop=True)
                s1 = lp.tile([128, 2 * BW], fp32, tag="s1")
                nc.scalar.activation(s1[:, :N], psa[:, :N], AF.Sqrt,
                                     bias=epsr, scale=1.0 / D)
                r1 = lp.tile([128, 2 * BW], fp32, tag="r1")
                nc.vector.reciprocal_approx_fast(out=r1[:, :N], in_=s1[:, :N])
                t = lp.tile([128, 2 * BW], fp32, tag="t")
                nc.vector.tensor_mul(t[:, :N], pairc[:, :N], r1[:, :N])

                psb = psB.tile([128, 2 * BW], fp32, tag="psB")
                nc.tensor.matmul(psb[:, :N], ct["fbd1p"], t[:, :N],
                                 start=True, stop=True)
                hr = lp.tile([128, 2 * BW], fp32, tag="hr")
                nc.scalar.activation(hr[:, :N], psb[:, :N], AF.Relu,
                                     bias=fb1r, scale=1.0)

                psc = psC.tile([128, 2 * BW], fp32, tag="psC")
                nc.tensor.matmul(psc[:, :N], ct["fbd2c"], hr[:, :N],
                                 start=True, stop=False)
                nc.tensor.matmul(psc[:, :N], ct["gbd"], t[:, :N],
                                 start=False, stop=True)

                ysq = lp.tile([128, 2 * BW], fp32, tag="ysq")
                nc.scalar.activation(ysq[:, :N], psc[:, :N], AF.Square,
                                     bias=ycb, scale=1.0)
                pse = psE.tile([128, 2 * BW], fp32, tag="psE")
                nc.tensor.matmul(pse[:, :N], ct["onesbd"], ysq[:, :N],
                                 start=True, stop=True)
                s2 = lp.tile([128, 2 * BW], fp32, tag="s2")
                nc.scalar.activation(s2[:, :N], pse[:, :N], AF.Sqrt,
                                     bias=epsr, scale=1.0 / D)
                r2 = lp.tile([128, 2 * BW], fp32, tag="r2")
                nc.vector.reciprocal_approx_fast(out=r2[:, :N], in_=s2[:, :N])
                u = lp.tile([128, 2 * BW], fp32, tag="u")
                nc.vector.scalar_tensor_tensor(
                    u[:, :N], psc[:, :N], ycb, r2[:, :N],
                    op0=OP.add, op1=OP.mult)

                psf = psF.tile([NB, 2 * BW], fp32, tag="psF")
                nc.tensor.matmul(psf[:, :N], ct["owp"], u[:, :N],
                                 start=True, stop=True)
                orow = outp.tile([NB, 2 * BW], fp32, tag="orow")
                for h in range(n_i):
                    nc.vector.scalar_tensor_tensor(
                        orow[:, h * BW:(h + 1) * BW],
                        psf[:, h * BW:(h + 1) * BW],
                        ct["ddgi"][:, i0 + h:i0 + h + 1],
                        ct["ddgjb"], op0=OP.add, op1=OP.add)

                nc.sync.dma_start(
                    out_d[i0:i0 + n_i, :].rearrange("h (b c) -> b h c", b=NB),
                    orow[:, :N].rearrange("b (h c) -> b h c", c=BW))

    nc.compile()
    return nc


def _host_prep(mut1_feat, mut1_ddg, aw1, ab1, aw2, ab2, ag, abt,
               hg, hbt, fw1, fb1, fw2, fb2, fg, fbt, ow, ob):
    f32 = np.float32
    C = (np.eye(D) - np.ones((D, D)) / D).astype(np.float64)
    bd = lambda m: np.kron(np.eye(NB), m).astype(f32)
    tile4 = lambda v: np.tile(np.asarray(v, np.float64), NB).astype(f32)

    fm = np.asarray(mut1_feat, np.float64).reshape(M, D)
    x4 = fm.reshape(NB, BW, D).transpose(0, 2, 1).reshape(128, BW).astype(f32)

    aw1_, aw2_, fw1_, fw2_, ow_ = [np.asarray(a, np.float64) for a in
                                   (aw1, aw2, fw1, fw2, ow)]
    ag_, abt_, hg_, hbt_, fg_, fbt_, ab1_, ab2_, fb1_, fb2_, ob_ = [
        np.asarray(a, np.float64) for a in
        (ag, abt, hg, hbt, fg, fbt, ab1, ab2, fb1, fb2, ob)]

    consts = {
        "x4": x4,
        "wbd1": bd(aw1_),
        "wbd2c": bd(aw2_ @ C),
        "ibdc": bd(C),
        "onesbd": bd(np.ones((D, D))),
        "agcbd": bd(ag_[:, None] * C),
        "fbd1p": bd(hg_[:, None] * fw1_),
        "fbd2c": bd(fw2_ @ C),
        "gbd": bd(hg_[:, None] * C),
        "owp": np.kron(np.eye(NB), (fg_ * ow_[:, 0])[:, None]).astype(f32),
        "vecs": np.stack([
            tile4(ab1_),
            tile4(ab2_ @ C),
            tile4(abt_ @ C),
            tile4(fb1_ + hbt_ @ fw1_),
            tile4((fb2_ + hbt_) @ C),
            np.full(128, EPS),
        ], axis=1).astype(f32),
    }
    ddg = np.asarray(mut1_ddg, np.float64).reshape(M)
    obp = float(ob_[0] + fbt_ @ ow_[:, 0])
    ddgjb = (ddg.reshape(NB, BW) + obp).astype(f32)

    wpack_base = np.zeros((128, WPW), f32)
    for k, (o, w) in WOFF.items():
        if k in consts:
            wpack_base[:, o:o + w] = consts[k]

    in_maps = []
    for c in range(NCORES):
        r0 = c * RPC
        b0, h = c // 2, c % 2
        sel = np.zeros((128, 128), f32)
        for b in range(NB):
            sel[32 * b0:32 * b0 + 32, 32 * b:32 * b + 32] = np.eye(D)
        wpack = wpack_base.copy()
        o, w = WOFF["pa" if h == 0 else "pb"]
        wpack[:, o:o + w] = sel
        dpack = np.zeros((NB, BW + RPC), f32)
        dpack[:, 0:BW] = ddgjb
        dpack[:, BW:] = np.tile(ddg[r0:r0 + RPC], (NB, 1))
        in_maps.append({"wpack": wpack, "dpack": dpack})
    return in_maps


def _run(in_maps, **kw):
    from concourse.bass_utils import run_bass_kernel_spmd
    if "nc" not in _CACHE:
        _CACHE["nc"] = _build_program()
    return run_bass_kernel_spmd(_CACHE["nc"], in_maps,
                                core_ids=list(range(NCORES)), **kw)


def kernel(**inputs):
    res = _run(_host_prep(**inputs))
    rows = np.concatenate([res.results[c]["out"] for c in range(NCORES)], axis=0)
    return rows.reshape(B, L, A, L, A).astype(np.float32)
